# revision 41
# baseline (speedup 1.0000x reference)
"""Single-head causal attention (B=8, T=4096, EMB=1024, HEAD=64) on 8 trn2 cores.

Strategy: data-parallel over batch, one batch element per NeuronCore.

The schedule is BAND-MAJOR: x arrives as four 1024-column bands (each band =
cols [1024b, 1024b+1024) of all 8 EMB-chunks), and every pipeline stage only
needs one band at a time, so compute starts when band 0 lands (~13us) instead
of when all of x lands (~31us):

  per band b (t-tiles j = 2b, 2b+1):
    1. KQ^T for the band's columns (k-outer over the 8 EMB chunks, 2 PSUM
       banks), drained to kq_sb = [K^T; Q^T]; SBUF->SBUF DMA builds the
       swapped qk2_sb = [Q^T; K^T] slice for PE row-tiling.
    2. V projection for row-blocks i in [8b, 8b+8) (xT band chunk stationary
       x Wv moving -> natural [t, 64] + ones column for the softmax rowsum).
    3. For every chunk a < 8b+8: the scores group S^T[a-block, band] via PE
       row-tiling (contraction d=64: even t-tiles in array rows 0:63, odd in
       64:127 -> 2 matmuls in flight), exp via ScalarE straight from PSUM
       (1/8 scale folded in, bf16 out), diag 128x128 block masked by one DVE
       0/1 multiply on the first band of each chunk.
    4. PV(i=a) for a in [8b, 8b+8): P^T tiles stationary, V+ones moving,
       accumulated in a no-start PSUM slot (DVE zeros + has_written
       accumulate), normalized (reciprocal rowsum) and DMA'd out.

Dummy matmuls on garbage SBUF during the initial DMA wait keep the PE HAM
clock-gate at 8/8 so the real pipeline never runs at the 1.2 GHz cold clock.
"""

from contextlib import ExitStack

import numpy as np
import ml_dtypes

B, T, EMB, HEAD = 8, 4096, 1024, 64
KCH = EMB // 128          # 8 contraction chunks
NTT = T // 512            # 8 t-tiles of 512
NTS = T // 128            # 32 t-subtiles / s-chunks of 128
NB = 4                    # column bands of 1024
BF16 = ml_dtypes.bfloat16

_CACHE = {}


def _build_program():
    import concourse.bacc as bacc
    import concourse.tile as tile
    from concourse import mybir

    fp32 = mybir.dt.float32
    bf16 = mybir.dt.bfloat16
    EXP = mybir.ActivationFunctionType.Exp

    nc = bacc.Bacc("TRN2", target_bir_lowering=False, debug=False)
    xt_ap = nc.dram_tensor("xt", [EMB, T], bf16, kind="ExternalInput").ap()
    # w2: [128, 1664] = per-partition-rearranged W (8 chunks x 192) ++ the
    # 0/1 upper-triangular diag mask [128,128] (see _host_prep)
    w_ap = nc.dram_tensor("w", [128, 1664], bf16, kind="ExternalInput").ap()
    o_ap = nc.dram_tensor("o", [T, HEAD], fp32, kind="ExternalOutput").ap()

    with tile.TileContext(nc) as tc:
        with (
            tc.tile_pool(name="consts", bufs=1) as consts,
            tc.tile_pool(name="outs", bufs=4) as outs,
        ):
            # x band tiles: xtH (bands 2,3) below xtL (bands 0,1) on the
            # right stack so xtL can release first (LIFO) when band 2's P
            # tiles need the space
            xHs = ExitStack()
            xpH = xHs.enter_context(tc.tile_pool(name="xpH", bufs=1, side="right"))
            xtH = xpH.tile([128, KCH, 2048], bf16, tag="xtH")
            xLs = ExitStack()
            xpL = xLs.enter_context(tc.tile_pool(name="xpL", bufs=1, side="right"))
            xtL = xpL.tile([128, KCH, 2048], bf16, tag="xtL")

            def xt_band(k, b, lo, hi):
                t0 = (b % 2) * 1024
                src = xtL if b < 2 else xtH
                return src[:, k, t0 + lo:t0 + hi]

            # ---- x band DMAs.  sync: bands 0,1; scalar: w2 then bands 2,3.
            # Each [128, 1024] instruction is 128 descriptors of 2KB on one
            # in-order ring; the 8 DMA-completion semaphore lanes recycle
            # against long-finished predecessors only.
            for k in range(7):
                nc.sync.dma_start(out=xt_band(k, 0, 0, 1024),
                                  in_=xt_ap[k * 128:(k + 1) * 128, 0:1024])
            w_sb = consts.tile([128, 1664], bf16, tag="w")
            nc.scalar.dma_start(out=w_sb, in_=w_ap)
            nc.sync.dma_start(out=xt_band(7, 0, 0, 1024),
                              in_=xt_ap[7 * 128:, 0:1024])
            for k in range(KCH):
                nc.sync.dma_start(out=xt_band(k, 1, 0, 1024),
                                  in_=xt_ap[k * 128:(k + 1) * 128, 1024:2048])
            for b in (2, 3):
                for k in range(KCH):
                    nc.scalar.dma_start(
                        out=xt_band(k, b, 0, 1024),
                        in_=xt_ap[k * 128:(k + 1) * 128, b * 1024:(b + 1) * 1024])

            # ---- constants / helpers
            warm = outs.tile([128, 1], fp32, tag="warm")
            nc.scalar.activation(warm, w_sb[:, 0:1], EXP)
            vt_sb = consts.tile([128, NTS, 65], bf16, tag="vt")
            nc.gpsimd.memset(vt_sb, 1.0)
            zeros_sb = consts.tile([128, 65], fp32, tag="zeros")
            nc.gpsimd.memset(zeros_sb, 0.0)
            mask_ap = w_sb[:, 1536:1664]   # mask[s, t] = (s <= t)
            kq_sb = consts.tile([128, T], bf16, tag="kq")    # [K^T; Q^T]
            qk2_sb = consts.tile([128, T], bf16, tag="qk2")  # [Q^T; K^T]

            phase = ExitStack()
            ptA = phase.enter_context(tc.tile_pool(name="ptA", bufs=1))
            ptB_pool = [None]
            ps_s = phase.enter_context(tc.tile_pool(name="ps_s", bufs=2, space="PSUM"))
            ps_kq = phase.enter_context(tc.tile_pool(name="ps_kq", bufs=1, space="PSUM"))
            ps_v = phase.enter_context(tc.tile_pool(name="ps_v", bufs=1, space="PSUM"))
            ps_o = phase.enter_context(tc.tile_pool(name="ps_o", bufs=1, space="PSUM"))
            pt = [None] * NTS
            po_sb = ps_o.tile([128, 4, 128], fp32, tag="po")

            def emit_kq_band(b, dummies=False):
                """KQ^T cols [1024b, 1024b+1024): k-outer, 2 PSUM banks,
                then PSUM->SBUF casts and the swapped qk2 copies."""
                pkq = ps_kq.tile([128, 1024], fp32, tag="kqb", name=f"kqb{b}")
                if dummies:
                    # ~7us of garbage matmuls so HAM is at the 2.4 GHz warm
                    # clock when band 0 lands; cleared by KQ's first start
                    for _ in range(26):
                        nc.tensor.matmul(
                            pkq[:, 0:512], kq_sb[:, 0:128], kq_sb[:, 0:512],
                            start=True, stop=True, skip_group_check=True,
                        )
                for k in range(KCH):
                    for jj in range(2):
                        nc.tensor.matmul(
                            pkq[:, jj * 512:(jj + 1) * 512],
                            w_sb[:, k * 192:k * 192 + 128],
                            xt_band(k, b, jj * 512, (jj + 1) * 512),
                            start=(k == 0),
                            stop=(k == KCH - 1),
                            skip_group_check=True,
                        )
                lo, hi = b * 1024, (b + 1) * 1024
                nc.vector.tensor_copy(kq_sb[:, lo:hi], pkq)
                nc.gpsimd.dma_start(out=qk2_sb[0:64, lo:hi],
                                    in_=kq_sb[64:128, lo:hi])
                nc.gpsimd.dma_start(out=qk2_sb[64:128, lo:hi],
                                    in_=kq_sb[0:64, lo:hi])

            vstate = {"blk": None}

            def emit_vquarter(q):
                """V projection rows i = 2q, 2q+1 (band q//4), k-inner."""
                if q % 4 == 0:
                    vstate["blk"] = ps_v.tile([128, 8, 64], fp32, tag="vblk",
                                              name=f"vblk{q // 4}")
                blk = vstate["blk"]
                for i in (2 * q, 2 * q + 1):
                    for k in range(KCH):
                        nc.tensor.matmul(
                            blk[:, i % 8, :],
                            xt_band(k, i // 8, (i % 8) * 128, (i % 8) * 128 + 128),
                            w_sb[:, k * 192 + 128:(k + 1) * 192],
                            start=(k == 0 and i % 8 == 0),
                            stop=(k == KCH - 1),
                            skip_group_check=True,
                        )
                if q % 4 == 3:
                    bb = q // 4
                    nc.vector.tensor_copy(vt_sb[:, 8 * bb:8 * bb + 8, 0:64], blk)

            def emit_scores(a, b):
                """The band-b scores group of chunk a: t-tiles
                [max(a//4, 2b) .. 2b+1], row-tiled by t-tile parity."""
                jstart = max(a // 4, 2 * b)
                g = 2 * b + 2 - jstart
                psg = ps_s.tile([128, 512 * g], fp32, tag="sg",
                                padded_shape=[128, 1024], name=f"sg{a}_{b}")
                for idx in range(g):
                    j = jstart + idx
                    if j % 2 == 0:
                        nc.tensor.matmul(
                            psg[:, idx * 512:(idx + 1) * 512],
                            kq_sb[0:64, a * 128:(a + 1) * 128],
                            qk2_sb[0:64, j * 512:(j + 1) * 512],
                            start=True, stop=True,
                        )
                    else:
                        nc.tensor.matmul(
                            psg[:, idx * 512:(idx + 1) * 512],
                            qk2_sb[64:128, a * 128:(a + 1) * 128],
                            kq_sb[64:128, j * 512:(j + 1) * 512],
                            start=True, stop=True,
                        )
                return jstart, g, psg

            def emit_exp(a, b, grp):
                jstart, g, psg = grp
                if pt[a] is None:
                    pool = ptA if a < 16 else ptB_pool[0]
                    pt[a] = pool.tile([128, T - 128 * a], bf16, tag=f"pt{a}",
                                      name=f"pt{a}")
                skip = max(0, 128 * a - 512 * jstart)
                out_lo = 512 * jstart + skip - 128 * a
                out_hi = 512 * (jstart + g) - 128 * a
                nc.scalar.activation(
                    pt[a][:, out_lo:out_hi], psg[:, skip:512 * g],
                    EXP, scale=0.125,
                )
                if jstart == a // 4:
                    # first band of this chunk: mask the diag 128x128 block
                    nc.vector.tensor_mul(pt[a][:, 0:128], pt[a][:, 0:128],
                                         mask_ap)

            def emit_pv(i):
                """PV for row-block i: no-start accumulation into a rotating
                quarter-bank slot (DVE zeros it; matmuls add via stale
                has_written bits or overwrite the zeros - both correct)."""
                slot = po_sb[:, i % 4, 0:65]
                nc.vector.tensor_copy(slot, zeros_sb)
                for aa in range(i + 1):
                    nc.tensor.matmul(
                        slot,
                        pt[aa][:, 128 * (i - aa):128 * (i - aa) + 128],
                        vt_sb[:, aa, :],
                        start=False,
                        stop=(aa == i),
                        skip_group_check=True,
                    )
                dr = outs.tile([128, 1], fp32, tag="dr", name=f"dr{i}")
                nc.vector.reciprocal(dr, slot[:, 64:65])
                o_sb = outs.tile([128, 64], fp32, tag="o_sb", name=f"osb{i}")
                nc.vector.tensor_scalar_mul(o_sb, slot[:, 0:64], dr)
                nc.sync.dma_start(out=o_ap[i * 128:(i + 1) * 128, :], in_=o_sb)

            # ---------- band-major pipeline ----------
            emit_kq_band(0, dummies=True)
            emit_vquarter(0)
            emit_vquarter(1)
            for b in range(NB):
                # this band's remaining V quarters (vt[i] needed by PV(i))
                emit_vquarter(4 * b + 2)
                emit_vquarter(4 * b + 3)
                if b == 2:
                    # bands 0,1 of x fully consumed (KQ + V done): release
                    # the tile so band-2/3 P tiles fit in SBUF
                    xLs.close()
                    ptB_pool[0] = phase.enter_context(
                        tc.tile_pool(name="ptB", bufs=1)
                    )
                grp = emit_scores(0, b)
                for a in range(8 * b + 8):
                    emit_exp(a, b, grp)
                    if a + 1 < 8 * b + 8:
                        grp = emit_scores(a + 1, b)
                    if a >= 8 * b:
                        emit_pv(a)
                    # interleave next band's KQ + first V quarters so they
                    # run on PE the moment the band's DMA lands
                    if a == 8 * b + 4 and b + 1 < NB:
                        emit_kq_band(b + 1)
                    if a == 8 * b + 6 and b + 1 < NB:
                        emit_vquarter(4 * b + 4)
                        emit_vquarter(4 * b + 5)
            phase.close()
            xHs.close()

    nc.compile()
    return nc


def _get_nc():
    if "nc" not in _CACHE:
        _CACHE["nc"] = _build_program()
    return _CACHE["nc"]


def _host_prep(x, W):
    """-> (xt [B, EMB, T] bf16, w2 [128, 1664] bf16)."""
    x = np.asarray(x, dtype=np.float32)
    W = np.asarray(W, dtype=np.float32)
    assert x.shape == (B, T, EMB) and W.shape == (EMB, 3 * HEAD)

    xt = np.ascontiguousarray(x.transpose(0, 2, 1)).astype(BF16)  # [B, EMB, T]
    # w2[p, k*192:(k+1)*192] = W[k*128+p, :]; then the 0/1 diag-block mask
    w2 = np.zeros((128, 1664), np.float32)
    w2[:, 0:1536] = W.reshape(KCH, 128, 192).transpose(1, 0, 2).reshape(128, 1536)
    w2[:, 1536:1664] = np.triu(np.ones((128, 128), np.float32))
    return xt, w2.astype(BF16)


def kernel(x, W):
    from concourse.bass_utils import run_bass_kernel_spmd

    xt, w2 = _host_prep(x, W)
    nc = _get_nc()
    in_maps = [{"xt": xt[b], "w": w2} for b in range(B)]
    res = run_bass_kernel_spmd(nc, in_maps, list(range(B)))
    return np.stack([res.results[b]["o"] for b in range(B)]).astype(np.float32)


# revision 42
# speedup vs baseline: 1.0867x; 1.0867x over previous
"""Single-head causal attention (B=8, T=4096, EMB=1024, HEAD=64) on 8 trn2 cores.

Strategy: data-parallel over batch, one batch element per NeuronCore.

The schedule is BAND-MAJOR: x arrives as four 1024-column bands (each band =
cols [1024b, 1024b+1024) of all 8 EMB-chunks), and every pipeline stage only
needs one band at a time, so compute starts when band 0 lands (~13us) instead
of when all of x lands (~31us):

  per band b (t-tiles j = 2b, 2b+1):
    1. KQ^T for the band's columns (k-outer over the 8 EMB chunks, 2 PSUM
       banks), drained to kq_sb = [K^T; Q^T]; SBUF->SBUF DMA builds the
       swapped qk2_sb = [Q^T; K^T] slice for PE row-tiling.
    2. V projection for row-blocks i in [8b, 8b+8) (xT band chunk stationary
       x Wv moving -> natural [t, 64] + ones column for the softmax rowsum).
    3. For every chunk a < 8b+8: the scores group S^T[a-block, band] via PE
       row-tiling (contraction d=64: even t-tiles in array rows 0:63, odd in
       64:127 -> 2 matmuls in flight), exp via ScalarE straight from PSUM
       (1/8 scale folded in, bf16 out), diag 128x128 block masked by one DVE
       0/1 multiply on the first band of each chunk.
    4. PV(i=a) for a in [8b, 8b+8): P^T tiles stationary, V+ones moving,
       accumulated in a no-start PSUM slot (DVE zeros + has_written
       accumulate), normalized (reciprocal rowsum) and DMA'd out.

Dummy matmuls on garbage SBUF during the initial DMA wait keep the PE HAM
clock-gate at 8/8 so the real pipeline never runs at the 1.2 GHz cold clock.
"""

from contextlib import ExitStack

import numpy as np
import ml_dtypes

B, T, EMB, HEAD = 8, 4096, 1024, 64
KCH = EMB // 128          # 8 contraction chunks
NTT = T // 512            # 8 t-tiles of 512
NTS = T // 128            # 32 t-subtiles / s-chunks of 128
NB = 4                    # column bands of 1024
BF16 = ml_dtypes.bfloat16

_CACHE = {}


def _build_program():
    import concourse.bacc as bacc
    import concourse.tile as tile
    from concourse import mybir

    fp32 = mybir.dt.float32
    bf16 = mybir.dt.bfloat16
    EXP = mybir.ActivationFunctionType.Exp

    nc = bacc.Bacc("TRN2", target_bir_lowering=False, debug=False)
    xt_ap = nc.dram_tensor("xt", [EMB, T], bf16, kind="ExternalInput").ap()
    # w2: [128, 1664] = per-partition-rearranged W (8 chunks x 192) ++ the
    # 0/1 upper-triangular diag mask [128,128] (see _host_prep)
    w_ap = nc.dram_tensor("w", [128, 1664], bf16, kind="ExternalInput").ap()
    o_ap = nc.dram_tensor("o", [T, HEAD], fp32, kind="ExternalOutput").ap()

    with tile.TileContext(nc) as tc:
        with (
            tc.tile_pool(name="consts", bufs=1) as consts,
            tc.tile_pool(name="outs", bufs=4) as outs,
        ):
            # x band tiles: xtH (bands 2,3) below xtL (bands 0,1) on the
            # right stack so xtL can release first (LIFO) when band 2's P
            # tiles need the space
            xHs = ExitStack()
            xpH = xHs.enter_context(tc.tile_pool(name="xpH", bufs=1, side="right"))
            xtH = xpH.tile([128, KCH, 2048], bf16, tag="xtH")
            xLs = ExitStack()
            xpL = xLs.enter_context(tc.tile_pool(name="xpL", bufs=1, side="right"))
            xtL = xpL.tile([128, KCH, 2048], bf16, tag="xtL")

            def xt_band(k, b, lo, hi):
                t0 = (b % 2) * 1024
                src = xtL if b < 2 else xtH
                return src[:, k, t0 + lo:t0 + hi]

            # ---- x band DMAs.  sync: bands 0,1; scalar: w2 then bands 2,3.
            # Each [128, 1024] instruction is 128 descriptors of 2KB on one
            # in-order ring; the 8 DMA-completion semaphore lanes recycle
            # against long-finished predecessors only.
            for k in range(7):
                nc.sync.dma_start(out=xt_band(k, 0, 0, 1024),
                                  in_=xt_ap[k * 128:(k + 1) * 128, 0:1024])
            w_sb = consts.tile([128, 1664], bf16, tag="w")
            nc.scalar.dma_start(out=w_sb, in_=w_ap)
            nc.sync.dma_start(out=xt_band(7, 0, 0, 1024),
                              in_=xt_ap[7 * 128:, 0:1024])
            for k in range(KCH):
                nc.sync.dma_start(out=xt_band(k, 1, 0, 1024),
                                  in_=xt_ap[k * 128:(k + 1) * 128, 1024:2048])
            for b in (2, 3):
                for k in range(KCH):
                    # sync, not scalar: DMA instructions on the scalar queue
                    # would sit ahead of every exp in the ACT FIFO
                    nc.sync.dma_start(
                        out=xt_band(k, b, 0, 1024),
                        in_=xt_ap[k * 128:(k + 1) * 128, b * 1024:(b + 1) * 1024])

            # ---- constants / helpers
            warm = outs.tile([128, 1], fp32, tag="warm")
            nc.scalar.activation(warm, w_sb[:, 0:1], EXP)
            vt_sb = consts.tile([128, NTS, 65], bf16, tag="vt")
            nc.gpsimd.memset(vt_sb, 1.0)
            zeros_sb = consts.tile([128, 65], fp32, tag="zeros")
            nc.gpsimd.memset(zeros_sb, 0.0)
            mask_ap = w_sb[:, 1536:1664]   # mask[s, t] = (s <= t)
            kq_sb = consts.tile([128, T], bf16, tag="kq")    # [K^T; Q^T]
            qk2_sb = consts.tile([128, T], bf16, tag="qk2")  # [Q^T; K^T]

            phase = ExitStack()
            ptA = phase.enter_context(tc.tile_pool(name="ptA", bufs=1))
            ptB_pool = [None]
            ps_s = phase.enter_context(tc.tile_pool(name="ps_s", bufs=2, space="PSUM"))
            ps_kq = phase.enter_context(tc.tile_pool(name="ps_kq", bufs=1, space="PSUM"))
            ps_v = phase.enter_context(tc.tile_pool(name="ps_v", bufs=1, space="PSUM"))
            ps_o = phase.enter_context(tc.tile_pool(name="ps_o", bufs=1, space="PSUM"))
            pt = [None] * NTS
            po_sb = ps_o.tile([128, 4, 128], fp32, tag="po")

            def emit_kq_band(b, dummies=False):
                """KQ^T cols [1024b, 1024b+1024): k-outer, 2 PSUM banks,
                then PSUM->SBUF casts and the swapped qk2 copies."""
                pkq = ps_kq.tile([128, 1024], fp32, tag="kqb", name=f"kqb{b}")
                if dummies:
                    # ~7us of garbage matmuls so HAM is at the 2.4 GHz warm
                    # clock when band 0 lands; cleared by KQ's first start
                    for _ in range(26):
                        nc.tensor.matmul(
                            pkq[:, 0:512], kq_sb[:, 0:128], kq_sb[:, 0:512],
                            start=True, stop=True, skip_group_check=True,
                        )
                for k in range(KCH):
                    for jj in range(2):
                        nc.tensor.matmul(
                            pkq[:, jj * 512:(jj + 1) * 512],
                            w_sb[:, k * 192:k * 192 + 128],
                            xt_band(k, b, jj * 512, (jj + 1) * 512),
                            start=(k == 0),
                            stop=(k == KCH - 1),
                            skip_group_check=True,
                        )
                lo, hi = b * 1024, (b + 1) * 1024
                nc.vector.tensor_copy(kq_sb[:, lo:hi], pkq)
                nc.gpsimd.dma_start(out=qk2_sb[0:64, lo:hi],
                                    in_=kq_sb[64:128, lo:hi])
                nc.gpsimd.dma_start(out=qk2_sb[64:128, lo:hi],
                                    in_=kq_sb[0:64, lo:hi])

            vstate = {"blk": None}

            def emit_vquarter(q):
                """V projection rows i = 2q, 2q+1 (band q//4), k-inner."""
                if q % 4 == 0:
                    vstate["blk"] = ps_v.tile([128, 8, 64], fp32, tag="vblk",
                                              name=f"vblk{q // 4}")
                blk = vstate["blk"]
                for i in (2 * q, 2 * q + 1):
                    for k in range(KCH):
                        nc.tensor.matmul(
                            blk[:, i % 8, :],
                            xt_band(k, i // 8, (i % 8) * 128, (i % 8) * 128 + 128),
                            w_sb[:, k * 192 + 128:(k + 1) * 192],
                            start=(k == 0 and i % 8 == 0),
                            stop=(k == KCH - 1),
                            skip_group_check=True,
                        )
                if q % 4 == 3:
                    bb = q // 4
                    nc.vector.tensor_copy(vt_sb[:, 8 * bb:8 * bb + 8, 0:64], blk)

            def emit_scores(a, b):
                """The band-b scores group of chunk a: t-tiles
                [max(a//4, 2b) .. 2b+1], row-tiled by t-tile parity."""
                jstart = max(a // 4, 2 * b)
                g = 2 * b + 2 - jstart
                psg = ps_s.tile([128, 512 * g], fp32, tag="sg",
                                padded_shape=[128, 1024], name=f"sg{a}_{b}")
                for idx in range(g):
                    j = jstart + idx
                    if j % 2 == 0:
                        nc.tensor.matmul(
                            psg[:, idx * 512:(idx + 1) * 512],
                            kq_sb[0:64, a * 128:(a + 1) * 128],
                            qk2_sb[0:64, j * 512:(j + 1) * 512],
                            start=True, stop=True,
                        )
                    else:
                        nc.tensor.matmul(
                            psg[:, idx * 512:(idx + 1) * 512],
                            qk2_sb[64:128, a * 128:(a + 1) * 128],
                            kq_sb[64:128, j * 512:(j + 1) * 512],
                            start=True, stop=True,
                        )
                return jstart, g, psg

            def emit_exp(a, b, grp):
                jstart, g, psg = grp
                if pt[a] is None:
                    pool = ptA if a < 16 else ptB_pool[0]
                    pt[a] = pool.tile([128, T - 128 * a], bf16, tag=f"pt{a}",
                                      name=f"pt{a}")
                skip = max(0, 128 * a - 512 * jstart)
                out_lo = 512 * jstart + skip - 128 * a
                out_hi = 512 * (jstart + g) - 128 * a
                nc.scalar.activation(
                    pt[a][:, out_lo:out_hi], psg[:, skip:512 * g],
                    EXP, scale=0.125,
                )
                if jstart == a // 4:
                    # first band of this chunk: mask the diag 128x128 block
                    nc.vector.tensor_mul(pt[a][:, 0:128], pt[a][:, 0:128],
                                         mask_ap)

            def emit_pv(i):
                """PV for row-block i: no-start accumulation into a rotating
                quarter-bank slot (DVE zeros it; matmuls add via stale
                has_written bits or overwrite the zeros - both correct)."""
                slot = po_sb[:, i % 4, 0:65]
                nc.vector.tensor_copy(slot, zeros_sb)
                for aa in range(i + 1):
                    nc.tensor.matmul(
                        slot,
                        pt[aa][:, 128 * (i - aa):128 * (i - aa) + 128],
                        vt_sb[:, aa, :],
                        start=False,
                        stop=(aa == i),
                        skip_group_check=True,
                    )
                dr = outs.tile([128, 1], fp32, tag="dr", name=f"dr{i}")
                nc.vector.reciprocal(dr, slot[:, 64:65])
                o_sb = outs.tile([128, 64], fp32, tag="o_sb", name=f"osb{i}")
                nc.vector.tensor_scalar_mul(o_sb, slot[:, 0:64], dr)
                nc.sync.dma_start(out=o_ap[i * 128:(i + 1) * 128, :], in_=o_sb)

            # ---------- band-major pipeline ----------
            emit_kq_band(0, dummies=True)
            emit_vquarter(0)
            emit_vquarter(1)
            for b in range(NB):
                # this band's remaining V quarters (vt[i] needed by PV(i))
                emit_vquarter(4 * b + 2)
                emit_vquarter(4 * b + 3)
                if b == 2:
                    # bands 0,1 of x fully consumed (KQ + V done): release
                    # the tile so band-2/3 P tiles fit in SBUF
                    xLs.close()
                    ptB_pool[0] = phase.enter_context(
                        tc.tile_pool(name="ptB", bufs=1)
                    )
                grp = emit_scores(0, b)
                for a in range(8 * b + 8):
                    emit_exp(a, b, grp)
                    if a + 1 < 8 * b + 8:
                        grp = emit_scores(a + 1, b)
                    if a >= 8 * b:
                        emit_pv(a)
                    # interleave next band's KQ + first V quarters so they
                    # run on PE the moment the band's DMA lands
                    if a == 8 * b + 4 and b + 1 < NB:
                        emit_kq_band(b + 1)
                    if a == 8 * b + 6 and b + 1 < NB:
                        emit_vquarter(4 * b + 4)
                        emit_vquarter(4 * b + 5)
            phase.close()
            xHs.close()

    nc.compile()
    return nc


def _get_nc():
    if "nc" not in _CACHE:
        _CACHE["nc"] = _build_program()
    return _CACHE["nc"]


def _host_prep(x, W):
    """-> (xt [B, EMB, T] bf16, w2 [128, 1664] bf16)."""
    x = np.asarray(x, dtype=np.float32)
    W = np.asarray(W, dtype=np.float32)
    assert x.shape == (B, T, EMB) and W.shape == (EMB, 3 * HEAD)

    xt = np.ascontiguousarray(x.transpose(0, 2, 1)).astype(BF16)  # [B, EMB, T]
    # w2[p, k*192:(k+1)*192] = W[k*128+p, :]; then the 0/1 diag-block mask
    w2 = np.zeros((128, 1664), np.float32)
    w2[:, 0:1536] = W.reshape(KCH, 128, 192).transpose(1, 0, 2).reshape(128, 1536)
    w2[:, 1536:1664] = np.triu(np.ones((128, 128), np.float32))
    return xt, w2.astype(BF16)


def kernel(x, W):
    from concourse.bass_utils import run_bass_kernel_spmd

    xt, w2 = _host_prep(x, W)
    nc = _get_nc()
    in_maps = [{"xt": xt[b], "w": w2} for b in range(B)]
    res = run_bass_kernel_spmd(nc, in_maps, list(range(B)))
    return np.stack([res.results[b]["o"] for b in range(B)]).astype(np.float32)
